# revision 3
# baseline (speedup 1.0000x reference)
"""AttnBlock (GroupNorm + single-head 1x1-conv attention + residual) on 8 TRN2 cores.

Sharding: core c handles batch b = c//2, query-token half c%2 (2048 of 4096
tokens). Each core computes GN + attention for its query half against all keys
of its batch element, returning [512, 2048]; host reassembles [4,512,64,64].

Algebraic folds (exact):
  - S = scale*(wq h + bq)@(wk h + bk): the bk term is constant along the
    softmax axis and cancels; the rest is h^T @ (Mt h + m) with
    Mt = scale*wk^T wq, m = scale*wk^T bq  -> K and Q never materialize.
  - wp folded into wv: M2 = wp wv, so the O accumulation (V P^T) directly
    produces projected output channels -> no separate projection matmuls.
  - v/p biases: wp@bv + bp folded into one per-channel bias bpp.

v2 changes vs v1 (HW: ~348us vs ~436us measured same-session; both engines'
micro-calibration showed exp/DVE match the CoreSim cost model while DR
matmuls run several times slower than modeled -- LDWEIGHTS dominates -- so
the schedule keeps all four engines loaded rather than chasing sim-optimal):
  - x streamed in as bf16 (halves the 8.4MB x DMA, split over the Pool+SP
    queues); GN stats + residual in bf16 are well inside the 2e-2 budget.
  - Mt/M2 quantized to fp8 on the host (drops the f32 staging DMA + DVE
    conversion copies); y written as bf16 (host upcasts).
  - GN stats split: tiles 0-2 on DVE bn_stats, tile 3 on the otherwise-idle
    Act engine via Copy/Square accum_out passes.
  - S tiles are [128, 2, 512] two-bank PSUM tiles: 4 DR matmuls fill both
    halves, ONE exp covers FD=1024 (halves Act per-instruction overhead).
  - Softmax denominator L: tiny fp8 ones-stationary DR matmul per pair into
    a persistent [1,512] PSUM bank (PE) instead of a DVE running sum; tried
    moving it back to DVE (v3) -> 392us, so PE wins on HW.
  - O accumulates channel chunks 0-1 in a single 2-bank psO tile trailing
    the exps by DEPTH pairs; chunks 2-3 re-stream the resident fp8 P tiles,
    interleaved into the next block's pair loop (PSUM: psS 4 + psO 2 +
    psL 2 = 8 banks).  Last block: cs2 rides the spare psL bank in-loop and
    only a 16-matmul cs3 pass remains in the tail.
  - Block 0's pairs interleave with the GN-apply/V/Q chunks they depend on
    (V/Q tiles zipped 1:1 with S pairs so the two psS slots alternate fast
    (exp) and slow (DVE copy) consumers); block 1 absorbs b0's leftovers.
"""

import numpy as np
import ml_dtypes

B, C, HW = 4, 512, 64
N = HW * HW            # 4096 tokens
NQ = N // 2            # 2048 query tokens per core
NT = C // 128          # 4 channel tiles
NJ = N // 128          # 32 key chunks
NP = NJ // 2           # 16 key chunk pairs (DoubleRow granularity)
NBLK = NQ // 512       # 4 query blocks of 512
NG = 32                # groups
EPS = 1e-6
SCALE = 1.0 / np.sqrt(C)
ALPHA_Q = 256.0        # fp8 scale on Mt (q-tilde path)
ALPHA_V = 32.0         # fp8 scale on M2 (v path)

_CACHE = {}


def _build_nc(reps=1):
    import contextlib
    import concourse.bass as bass
    import concourse.mybir as mybir
    import concourse.tile as tile
    import concourse.bacc as bacc

    f32 = mybir.dt.float32
    f32r = mybir.dt.float32r
    bf16 = mybir.dt.bfloat16
    f8 = mybir.dt.float8e4
    f8e5 = mybir.dt.float8e5
    AF = mybir.ActivationFunctionType
    OP = mybir.AluOpType
    DR = mybir.MatmulPerfMode.DoubleRow
    DRS = mybir.MatmulPerfMode.DoubleRowSwInterleave

    nc = bacc.Bacc("TRN2", target_bir_lowering=False, debug=False, num_devices=8)

    x_d = nc.dram_tensor("x", [C, N], bf16, kind="ExternalInput").ap()
    mt8_d = nc.dram_tensor("mt8", [128, 2, NT, 256], f8, kind="ExternalInput").ap()
    wv8_d = nc.dram_tensor("wv8", [128, NT, C], f8, kind="ExternalInput").ap()
    mvec_d = nc.dram_tensor("mvec", [128, NT], f32, kind="ExternalInput").ap()
    bpp_d = nc.dram_tensor("bpp", [128, NT], f32, kind="ExternalInput").ap()
    gnsc_d = nc.dram_tensor("gnsc", [128, NT], f32, kind="ExternalInput").ap()
    gnbi_d = nc.dram_tensor("gnbi", [128, NT], f32, kind="ExternalInput").ap()
    ind_d = nc.dram_tensor("ind", [C, NG], f32, kind="ExternalInput").ap()
    emat_d = nc.dram_tensor("emat", [NG, 128], f32, kind="ExternalInput").ap()
    tmask_d = nc.dram_tensor("tmask", [NG, NT], f32, kind="ExternalInput").ap()
    ones_d = nc.dram_tensor("ones", [128, 2, 16], f8e5, kind="ExternalInput").ap()
    y_d = nc.dram_tensor("y", [C, NQ], bf16, kind="ExternalOutput").ap()

    # Each core gets its own x with its query half rolled to tokens
    # [0, 2048); key-side sums run over all N tokens so the roll is exact.

    x_t = x_d.rearrange("(t p) n -> t p n", p=128)
    y_t = y_d.rearrange("(t p) n -> t p n", p=128)

    with tile.TileContext(nc) as tc:
        with (
            tc.tile_pool(name="xpool", bufs=1) as xpool,
            tc.tile_pool(name="h8p", bufs=1) as h8p,
            tc.tile_pool(name="q8p", bufs=1) as q8p,
            tc.tile_pool(name="vpool", bufs=1) as vpool,
            tc.tile_pool(name="wpool", bufs=1) as wpool,
            tc.tile_pool(name="cpool", bufs=1) as cpool,
            tc.tile_pool(name="gn", bufs=1) as gn,
            tc.tile_pool(name="pt", bufs=24) as ptp,
            tc.tile_pool(name="fin", bufs=1) as finp,
            tc.tile_pool(name="psS", bufs=2, space="PSUM") as psS,
            tc.tile_pool(name="psO", bufs=1, space="PSUM") as psO,
            tc.tile_pool(name="psL", bufs=2, space="PSUM") as psL,
        ):
            with (tc.For_i(0, reps, 1) if reps > 1 else contextlib.nullcontext()):
                # ---- load x (stays resident bf16: GN stats, residual) ----
                # x chunks stream in on two queues (Pool + SP, both otherwise
                # idle here); tile 3 lands first on SP because the Act engine
                # computes its stats and would otherwise wait the longest.
                x = []
                for t in range(NT):
                    xt = xpool.tile([128, N], bf16, name=f"x{t}", tag=f"x{t}")
                    x.append(xt)
                for s in range(4):
                    nc.sync.dma_start(x[3][:, s * 1024:(s + 1) * 1024],
                                      x_t[3][:, s * 1024:(s + 1) * 1024])
                for t in range(3):
                    for s in range(4):
                        nc.gpsimd.dma_start(x[t][:, s * 1024:(s + 1) * 1024],
                                            x_t[t][:, s * 1024:(s + 1) * 1024])

                # ---- load weights (already fp8) and constants ----
                mt8 = wpool.tile([128, 2, NT, 256], f8, tag="mt8")
                wv8 = wpool.tile([128, NT, C], f8, tag="wv8")
                nc.gpsimd.dma_start(mt8[:], mt8_d[:])
                nc.gpsimd.dma_start(wv8[:], wv8_d[:])

                mvec_sb = cpool.tile([128, NT], f32)
                bpp_sb = cpool.tile([128, NT], f32)
                gnsc_sb = cpool.tile([128, NT], f32)
                gnbi_sb = cpool.tile([128, NT], f32)
                ind_sb = cpool.tile([128, NT, NG], f32)
                emat_sb = cpool.tile([NG, 128], f32)
                tmask_sb = cpool.tile([NG, NT], f32)
                ones_sb = cpool.tile([128, 2, 16], f8e5)
                nc.sync.dma_start(mvec_sb[:], mvec_d[:])
                nc.sync.dma_start(bpp_sb[:], bpp_d[:])
                nc.sync.dma_start(gnsc_sb[:], gnsc_d[:])
                nc.sync.dma_start(gnbi_sb[:], gnbi_d[:])
                nc.sync.dma_start(ind_sb[:], ind_d.rearrange("(t p) g -> p t g", p=128))
                nc.sync.dma_start(emat_sb[:], emat_d[:])
                nc.sync.dma_start(tmask_sb[:], tmask_d[:])
                nc.sync.dma_start(ones_sb[:], ones_d[:])

                # ---- GroupNorm stats ----
                # Tiles 0-2: per-channel mean/var via DVE bn_stats (512-wide
                # chunks, HW max) -> stats3 = [mean, var, mean^2].
                # Tile 3: Act-engine accumulate (Copy and Square passes with
                # accum_out) -> stats3 = [mean, E[x^2], 0].  The group
                # aggregation only uses slot1+slot2 = E[x^2], so both forms
                # feed the same ind-matmul.
                eps_t = gn.tile([NG, 1], f32)
                nc.vector.memset(eps_t[:], EPS)
                stats3 = []
                for t in range(3):
                    bnb = gn.tile([128, 8, 6], f32, name=f"bnb{t}", tag="bnb", bufs=2)
                    for s in range(8):
                        nc.vector.bn_stats(
                            bnb[:, s, :], x[t][:, s * 512:(s + 1) * 512]
                        )
                    mv = gn.tile([128, 2], f32, name=f"mv{t}", tag="mv", bufs=2)
                    nc.vector.bn_aggr(mv[:], bnb[:])
                    s3 = gn.tile([128, 3], f32, name=f"s3_{t}", tag=f"s3_{t}")
                    nc.vector.tensor_copy(s3[:, 0:2], mv[:])
                    nc.scalar.square(s3[:, 2:3], mv[:, 0:1])
                    stats3.append(s3)
                s3a = gn.tile([128, 3], f32, name="s3_3", tag="s3_3")
                nc.vector.memset(s3a[:, 2:3], 0.0)
                scr = gn.tile([128, N], bf16, name="scr", tag="scr")
                nc.scalar.activation(scr[:], x[3][:], AF.Copy,
                                     scale=1.0 / N, accum_out=s3a[:, 0:1])
                nc.scalar.activation(scr[:], x[3][:], AF.Square,
                                     scale=1.0 / HW, accum_out=s3a[:, 1:2])
                stats3.append(s3a)
                # group aggregation: [32, 3] = sum_c ind[c, g] * [mean, var, mean^2]
                ps_g = psS.tile([NG, 3], f32, tag="psS")
                for t in range(NT):
                    nc.tensor.matmul(ps_g[:], ind_sb[:, t, :], stats3[t][:],
                                     start=(t == 0), stop=(t == NT - 1))
                # var_g = avg_var + avg_mean2 - avg_mean^2 ; inv = 1/sqrt(var+eps)
                sg = gn.tile([NG, 3], f32)
                nc.vector.tensor_copy(sg[:], ps_g[:])
                msq = gn.tile([NG, 1], f32)
                nc.scalar.square(msq[:], sg[:, 0:1])
                vg = gn.tile([NG, 1], f32)
                nc.vector.scalar_tensor_tensor(vg[:], sg[:, 1:2], msq[:],
                                               sg[:, 2:3], OP.subtract, OP.add)
                std = gn.tile([NG, 1], f32)
                nc.scalar.activation(std[:], vg[:], AF.Sqrt, bias=eps_t[:])
                inv = gn.tile([NG, 1], f32)
                nc.vector.reciprocal(inv[:], std[:])
                mcol = sg[:, 0:1]
                # spread group values back to channel layout via E-matmul
                rmat = gn.tile([NG, 2 * NT], f32)
                nc.vector.tensor_scalar_mul(rmat[:, 0:NT], tmask_sb[:], inv[:])
                nc.vector.tensor_scalar_mul(rmat[:, NT:2 * NT], tmask_sb[:], mcol[:])
                ps_e = psS.tile([128, 2 * NT], f32, tag="psS")
                nc.tensor.matmul(ps_e[:], emat_sb[:], rmat[:], start=True, stop=True)
                a_pc = gn.tile([128, NT], f32)
                b_pc = gn.tile([128, NT], f32)
                nc.vector.tensor_mul(a_pc[:], gnsc_sb[:], ps_e[:, 0:NT])
                nc.vector.tensor_mul(b_pc[:], ps_e[:, NT:2 * NT], a_pc[:])
                nc.vector.tensor_sub(b_pc[:], gnbi_sb[:], b_pc[:])

                # ---- GN apply (-> fp8 h8, 1024-token chunks) interleaved
                # with V-tilde (all tokens) and Q-tilde (query half only) ----
                h8 = h8p.tile([128, NT, N], f8, tag="h8")
                qt8 = q8p.tile([128, NT, NQ], f8, tag="qt8")
                vt8 = [vpool.tile([128, 2, C], f8, name=f"vt{g}", tag=f"vt{g}")
                       for g in range(NP)]
                def apply_s(s):
                    sel = slice(s * 1024, (s + 1) * 1024)
                    for t in range(NT):
                        nc.vector.tensor_scalar(
                            h8[:, t, sel], x[t][:, sel],
                            a_pc[:, t:t + 1], b_pc[:, t:t + 1],
                            OP.mult, OP.add)

                def emit_v1(g):
                    # one pair of key chunks -> one 2-bank tile
                    ps_v = psS.tile([128, 2, C], f32, tag="psS",
                                    name=f"ps_v{g}")
                    for j in range(2):
                        js = 2 * g + j
                        jsel = slice(js * 128, (js + 1) * 128)
                        for i in range(2):
                            nc.tensor.matmul(ps_v[:, j, :],
                                             h8[:, 2 * i:2 * i + 2, jsel],
                                             wv8[:, 2 * i:2 * i + 2, :],
                                             start=(i == 0), stop=(i == 1),
                                             perf_mode=DR)
                    if g < 8:  # early copies on Act (DVE is stats-busy),
                        nc.scalar.copy(vt8[g][:], ps_v[:])
                    else:      # late ones on DVE (Act is exp-saturated)
                        nc.vector.tensor_copy(vt8[g][:], ps_v[:])

                def emit_q1(s, co):
                    qsel = slice(s * 1024, (s + 1) * 1024)
                    ps_q = psS.tile([128, 2, 512], f32, tag="psS",
                                    name=f"ps_q{s}_{co}")
                    for half in range(2):
                        hsel = slice(s * 1024 + half * 512,
                                     s * 1024 + half * 512 + 512)
                        for i in range(2):
                            nc.tensor.matmul(
                                ps_q[:, half, :],
                                mt8[:, i, co, :],
                                h8[:, 2 * i:2 * i + 2, hsel],
                                start=(i == 0), stop=(i == 1),
                                perf_mode=DRS)
                    nc.vector.tensor_scalar_add(
                        qt8[:, co, qsel], ps_q[:],
                        mvec_sb[:, co:co + 1])

                # ---- attention over 4 query blocks of 512 ----
                # One flattened stream. Per pair: S (PE, 4 DR matmuls into a
                # 2-bank tile) -> exp (Act, FD=1024) -> resident fp8 P tile.
                # Trailing by DEPTH pairs: a tiny fp8 ones-matmul accumulates
                # the softmax denominator L in a persistent [1,512] PSUM bank
                # (PE instead of DVE), and O accumulates channel chunks 0-1 in
                # the single 2-bank psO tile. Channel chunks 2-3 of block b
                # re-stream b's resident P tiles into the recycled psO tile,
                # interleaved into the start of block b+1's pair loop so the
                # Act engine never drains. PSUM: psS 4 + psO 2 + psL 2 = 8.
                DEPTH = 6
                blk = {}

                def fin(ib, cs, src):
                    st = blk[ib]
                    nc.vector.scalar_tensor_tensor(
                        st["out"][:, cs, :], src, 1.0 / ALPHA_V,
                        st["lb"][:], OP.mult, OP.mult)
                    nc.vector.scalar_tensor_tensor(
                        st["out"][:, cs, :], st["out"][:, cs, :],
                        bpp_sb[:, cs:cs + 1], x[cs][:, st["isel"]],
                        OP.add, OP.add)

                def emit_o23(ib, pool=None, tag="psO"):
                    # fresh psO allocation (same single slot -> ordered after
                    # the fins that read this block's chunk 0-1 results)
                    st = blk[ib]
                    cs_list = st.get("o23_cs", (2, 3))
                    st["o"] = (pool or psO).tile([128, len(cs_list), 512], f32,
                                                 name=f"o23_{ib}", tag=tag)
                    for g in range(NP):
                        for k, cs in enumerate(cs_list):
                            nc.tensor.matmul(st["o"][:, k, :],
                                             vt8[g][:, :, cs * 128:
                                                   (cs + 1) * 128],
                                             st["pts"][g][:],
                                             start=(g == 0), stop=(g == NP - 1),
                                             perf_mode=DR)

                def finish_block(ib):
                    st = blk[ib]
                    for k, cs in enumerate(st.get("o23_cs", (2, 3))):
                        fin(ib, cs, st["o"][:, k, :])
                    nc.sync.dma_start(
                        y_t[2:4, :, st["isel"]].rearrange("t p n -> p t n"),
                        st["out"][:, 2:4, :])
                    del st["pts"], st["o"]

                def emit_ol(ib, g):
                    st = blk[ib]
                    last = ib == NBLK - 1
                    if g == 0:  # lazy alloc keeps psO/psL slot order == write order
                        st["o"] = psO.tile([128, 2, 512], f32, name=f"o{ib}",
                                           tag="psO")
                        st["l"] = psL.tile([1, 512], f32, name=f"l{ib}",
                                           tag="psL")
                        if last:
                            # last block: cs2 rides in the spare psL bank,
                            # trailing inside the loop, so the tail subpass
                            # only has cs3 left
                            st["o2"] = psL.tile([128, 512], f32,
                                                name=f"o2_{ib}", tag="psL")
                            st["o23_cs"] = (3,)
                    pt = st["pts"][g]
                    nc.tensor.matmul(st["l"][:], ones_sb[:, :, 0:1], pt[:],
                                     start=(g == 0), stop=(g == NP - 1),
                                     perf_mode=DR)
                    for k in range(2):
                        nc.tensor.matmul(st["o"][:, k, :],
                                         vt8[g][:, :, k * 128:(k + 1) * 128],
                                         pt[:],
                                         start=(g == 0), stop=(g == NP - 1),
                                         perf_mode=DR)
                    if last:
                        nc.tensor.matmul(st["o2"][:], vt8[g][:, :, 256:384],
                                         pt[:],
                                         start=(g == 0), stop=(g == NP - 1),
                                         perf_mode=DR)
                    if g == NP - 1:
                        lrec = finp.tile([1, 512], f32, tag="lrec", bufs=2,
                                         name=f"lrec{ib}")
                        nc.vector.reciprocal(lrec[:], st["l"][:])
                        lb = finp.tile([128, 512], f32, tag="lb", bufs=2,
                                       name=f"lb{ib}")
                        nc.gpsimd.partition_broadcast(lb[:], lrec[:])
                        st["lb"] = lb
                        st["out"] = finp.tile([128, NT, 512], bf16, tag="out",
                                              bufs=2, name=f"out{ib}")
                        fin(ib, 0, st["o"][:, 0, :])
                        fin(ib, 1, st["o"][:, 1, :])
                        nc.sync.dma_start(
                            y_t[0:2, :, st["isel"]].rearrange("t p n -> p t n"),
                            st["out"][:, 0:2, :])
                        if last:
                            fin(ib, 2, st["o2"][:])

                def emit_pair(ib, g):
                    st = blk[ib]
                    pt = ptp.tile([128, 2, 512], f8e5, tag="pt",
                                  name=f"pt{ib}_{g}")
                    ps_s = psS.tile([128, 2, 512], f32, tag="psS",
                                    name=f"ps_s{ib}_{g}")
                    for i in range(2):
                        js = 2 * g + i
                        jsel = slice(js * 128, (js + 1) * 128)
                        for ii in range(2):
                            nc.tensor.matmul(
                                ps_s[:, i, :], h8[:, 2 * ii:2 * ii + 2, jsel],
                                qt8[:, 2 * ii:2 * ii + 2, st["isel"]],
                                start=(ii == 0), stop=(ii == 1), perf_mode=DR)
                    nc.scalar.activation(pt[:], ps_s[:], AF.Exp,
                                         scale=1.0 / ALPHA_Q)
                    st["pts"].append(pt)
                    if g == DEPTH - 1 and ib > 0:
                        # previous block's channel chunks 2-3, placed here
                        # so Act keeps streaming this block's exps
                        emit_o23(ib - 1)
                    if g >= DEPTH:
                        emit_ol(ib, g - DEPTH)
                    if g == DEPTH + 2 and ib > 0:
                        finish_block(ib - 1)

                # Block 0's pairs interleave with the GN apply / V / Q chunks
                # they depend on, so Act starts exp-ing as soon as the first
                # 1024 tokens are normalized.  V/Q tiles are zipped one-for-one
                # with S pairs: the two psS slots then alternate between a
                # slow consumer (DVE copy / qt-add) and a fast one (Act exp),
                # instead of two V tiles locking PE to the DVE copy cadence.
                blk[0] = {"isel": slice(0, 512), "pts": []}
                apply_s(0)
                for co in range(NT):
                    emit_q1(0, co)
                for g in range(0, 4):
                    emit_v1(g)
                    emit_pair(0, g)
                apply_s(1)
                for g in range(4, 8):
                    emit_v1(g)
                    emit_pair(0, g)
                apply_s(2)
                for g in range(8, 12):
                    emit_v1(g)
                    emit_pair(0, g)
                apply_s(3)
                emit_v1(12)
                emit_pair(0, 12)
                emit_v1(13)
                emit_pair(0, 13)
                emit_pair(0, 14)
                emit_pair(0, 15)
                # block 1 absorbs b0's leftovers (V pairs 14-15, Q(s1) -- only
                # needed from block 2 on -- and b0's trailing O/L work) into
                # its early pairs, keeping the DVE cadence under the exp rate.
                blk[1] = {"isel": slice(512, 1024), "pts": []}
                pre1 = {0: ("v", 14), 1: ("v", 15),
                        2: ("q", 0), 3: ("q", 1), 4: ("q", 2), 5: ("q", 3)}
                post1 = {0: [10, 11], 1: [12, 13], 2: [14, 15]}
                for g in range(NP):
                    w = pre1.get(g)
                    if w is not None:
                        if w[0] == "v":
                            emit_v1(w[1])
                        else:
                            emit_q1(1, w[1])
                    emit_pair(1, g)
                    for gg in post1.get(g, []):
                        emit_ol(0, gg)
                for g in range(NP - DEPTH, NP):
                    emit_ol(1, g)
                for ib in range(2, NBLK):
                    blk[ib] = {
                        "isel": slice(ib * 512, (ib + 1) * 512),
                        "pts": [],
                    }
                    for g in range(NP):
                        emit_pair(ib, g)
                    for g in range(NP - DEPTH, NP):
                        emit_ol(ib, g)
                # last block's chunk 2-3 pass goes into a freed psS slot so it
                # runs concurrently with the L->recip->fin chain on the psO tile
                emit_o23(NBLK - 1, pool=psS, tag="psS")
                finish_block(NBLK - 1)
    nc.compile()
    return nc


def _host_prep(gn_scale, gn_bias, wq, bq, wk, bk, wv, bv, wp, bp):
    f = np.float32

    def pc(v):  # [512] -> [128, 4] channel layout (c = t*128 + p)
        return np.ascontiguousarray(v.reshape(NT, 128).T).astype(f)

    def plc8(m):  # [512, 512] -> [128, 4, 512] channel layout on dim 0, fp8
        a = np.ascontiguousarray(m.reshape(NT, 128, C).transpose(1, 0, 2)).astype(f)
        return np.clip(a, -448.0, 448.0).astype(ml_dtypes.float8_e4m3)

    wq64, wk64, wv64, wp64 = (np.asarray(w, np.float64) for w in (wq, wk, wv, wp))
    mt = SCALE * (wq64.T @ wk64)                                  # [c_in, c_out]
    m2 = wp64 @ wv64                                              # [c_out, c_in] -> used transposed
    mvec = pc((ALPHA_Q * SCALE * (wk64.T @ np.asarray(bq, np.float64))).astype(f))
    bpp = pc((np.asarray(bp, np.float64) + wp64 @ np.asarray(bv, np.float64)).astype(f))

    ind = np.zeros((C, NG), f)
    ind[np.arange(C), np.arange(C) // 16] = 1.0 / 16.0
    emat = np.zeros((NG, 128), f)
    for g in range(NG):
        for p in range(128):
            if p // 16 == g % 8:
                emat[g, p] = 1.0
    tmask = np.zeros((NG, NT), f)
    for g in range(NG):
        tmask[g, g // 8] = 1.0
    ones = np.ones((128, 2, 16), ml_dtypes.float8_e5m2)

    a = np.ascontiguousarray((ALPHA_Q * mt).reshape(NT, 128, C)
                             .transpose(1, 0, 2)).astype(f)
    a = np.clip(a, -448.0, 448.0)
    # DoubleRowSwInterleave weight layout: flat[2k+b] = (plane b, col 127-k)
    tmp = a.reshape(128, 2, 2, NT, 128)[:, :, :, :, ::-1]   # [p,i,b,co,k]
    mt8s = np.ascontiguousarray(tmp.transpose(0, 1, 3, 4, 2)
                                ).reshape(128, 2, NT, 256)
    return dict(
        ones=ones,
        mt8=mt8s.astype(ml_dtypes.float8_e4m3), wv8=plc8(ALPHA_V * m2.T),
        mvec=mvec, bpp=bpp,
        gnsc=pc(np.asarray(gn_scale, f)), gnbi=pc(np.asarray(gn_bias, f)),
        ind=ind, emat=emat, tmask=tmask,
    )


def _in_maps(hidden_states, shared):
    x = np.asarray(hidden_states, np.float32).reshape(B, C, N)
    in_maps = []
    for c in range(8):
        b, half = c // 2, c % 2
        xb = x[b]
        if half:
            # roll so this core's query tokens sit at [0, 2048)
            xb = np.concatenate([xb[:, NQ:], xb[:, :NQ]], axis=1)
        m = dict(shared)
        m["x"] = np.ascontiguousarray(xb).astype(ml_dtypes.bfloat16)
        in_maps.append(m)
    return in_maps


def kernel(hidden_states, gn_scale, gn_bias, wq, bq, wk, bk, wv, bv, wp, bp):
    from concourse.bass_utils import run_bass_kernel_spmd

    if "nc" not in _CACHE:
        _CACHE["nc"] = _build_nc()
    nc = _CACHE["nc"]

    shared = _host_prep(gn_scale, gn_bias, wq, bq, wk, bk, wv, bv, wp, bp)
    in_maps = _in_maps(hidden_states, shared)

    res = run_bass_kernel_spmd(nc, in_maps, list(range(8)))

    out = np.empty((B, C, N), np.float32)
    for c in range(8):
        b, half = c // 2, c % 2
        out[b][:, half * NQ:(half + 1) * NQ] = res.results[c]["y"].astype(np.float32)
    return out.reshape(B, C, HW, HW)
